# revision 1
# baseline (speedup 1.0000x reference)
"""LocalAttention1d Trainium2 kernel.

Math note: the reference applies softmax over a singleton axis
(softmax(a_t[..., None], axis=2)), which is exactly 1.0 for finite scores,
so the Luong-score path (the two big einsums over w_a) cancels out of the
output. The output reduces exactly to

    s_t[b, q] = sum_w exp(-s_exp[b, w]) * q_i[b, q, p[b] - 128 + w]

with p = round(p_t) from the predictive-alignment network, provided the
window [p-128, p+128) stays in bounds (guaranteed by the tiny v_p init; we
assert it). The tiny predictive network (c_t @ w_p.T -> tanh -> @ v_p.T ->
sigmoid, ~0.1% of the FLOPs) is evaluated on host in float64 to pick the
integer window positions; everything heavy (windowed gather of q_i and the
gaussian-weighted reduction) runs on the NeuronCores, data-parallel over
the batch dim (8 batches per core).

Device strategy (one fully static, branch-free NEFF run SPMD on 8 cores):
batches are assigned to (core, slot) by sorting on window position — slot
i holds sorted ranks [8i, 8i+8), one per core — so the 8 windows sharing a
slot nearly coincide. Each slot gets a static HWDGE DMA [q%128, q//128,
EW_i] at column A_i = min start (64-aligned), EW_i = spread + window,
covering every core's window for that slot. The gaussian weights arrive
zero-padded into the EW_i frame at each batch's offset, so a fused
multiply+reduce (custom DVE op affine_mul_reduce) over the full frame
yields the exact window sum (zero weights add exactly 0.0 in f32). The
[128, 64] accumulator goes out raw; the host untangles and unpermutes.
"""

import numpy as np

B, Q, N = 64, 1024, 2048
WIN = 256
HALF = WIN // 2  # 128
NCORES = 8
BL = B // NCORES  # batch slots per core
QC = Q // 128     # q chunks of 128
ALIGN = 16        # window start alignment (64B dma alignment)

_NC_CACHE = {}


def _build_nc(slot_geom):
    """slot_geom: tuple of (A_i, EW_i) per slot, baked into the NEFF."""
    import concourse.bass as bass
    import concourse.tile as tile
    from concourse import bacc, mybir

    f32 = mybir.dt.float32
    ew_max = max(ew for _, ew in slot_geom)
    nc = bacc.Bacc(
        "TRN2", target_bir_lowering=False, debug=False, num_devices=NCORES
    )
    qs = nc.dram_tensor("qs", [BL, Q, N], f32, kind="ExternalInput")
    gb = nc.dram_tensor("gb", [BL, ew_max], f32, kind="ExternalInput")
    # raw accumulator layout [q%128, slot*QC + qc]; host untangles it
    out = nc.dram_tensor("out", [128, BL * QC], f32, kind="ExternalOutput")

    # [128, BL, QC, N]: partition = q % 128, free = (slot, q-chunk, col)
    qsa = qs.ap().rearrange("i (qc p) n -> p i qc n", p=128)

    with tile.TileContext(nc) as tc:
        with (
            tc.tile_pool(name="small", bufs=1) as small,
            tc.tile_pool(name="wpool", bufs=BL) as wpool,
            tc.tile_pool(name="ppool", bufs=4) as ppool,
            tc.tile_pool(name="gpsum", bufs=BL, space="PSUM") as gpsum,
        ):
            # gaussian weights: load one row per slot, broadcast to 128
            # partitions with a ones-outer-product on the idle TensorEngine
            g_sb = small.tile([1, BL, ew_max], f32)
            nc.gpsimd.dma_start(g_sb, gb.ap().rearrange("i e -> (i e)")[None, :])
            ones = small.tile([1, 128], f32)
            nc.vector.memset(ones[:, :], 1.0)

            gts = []
            for i in range(BL):
                _, ew_i = slot_geom[i]
                gt = gpsum.tile([128, ew_max], f32, tag="gp")
                nc.tensor.matmul(
                    gt[:, :ew_i], ones[:, :], g_sb[0:1, i, :ew_i]
                )
                gts.append(gt)

            acc = small.tile([128, BL * QC], f32)

            wins = []
            for i in range(BL):
                a_i, ew_i = slot_geom[i]
                win = wpool.tile([128, QC, ew_max], f32, tag="win")
                src = qsa[:, i][:, :, a_i : a_i + ew_i]  # [128, QC, EW_i]
                engs = [nc.sync, nc.scalar, nc.gpsimd]
                e0 = engs[i % 3]
                e1 = engs[(i + 1) % 3]
                e2 = engs[(i + 2) % 3]
                e0.dma_start(win[:, 0:3, :ew_i], src[:, 0:3])
                e1.dma_start(win[:, 3:6, :ew_i], src[:, 3:6])
                e2.dma_start(win[:, 6:8, :ew_i], src[:, 6:8])
                wins.append(win)

            for i in range(BL):
                _, ew_i = slot_geom[i]
                for qc in range(QC):
                    prod = ppool.tile([128, ew_max], f32, tag="prod")
                    nc.vector.affine_mul_reduce(
                        out=prod[:, :ew_i],
                        accum_out=acc[:, i * QC + qc : i * QC + qc + 1],
                        in0=wins[i][:, qc, :ew_i],
                        in1=gts[i][:, :ew_i],
                        scale=1.0,
                        bias=0.0,
                    )

            nc.gpsimd.dma_start(out.ap(), acc[:, :])
    nc.compile()
    return nc


def _get_nc(slot_geom):
    key = tuple(slot_geom)
    if key not in _NC_CACHE:
        _NC_CACHE[key] = _build_nc(key)
    return _NC_CACHE[key]


def _predict_host(c_t, w_p, v_p):
    """float64 replica of sigmoid(tanh(c_t @ w_p.T) @ v_p.T) * (N+1-2)."""
    z = np.tanh(c_t.astype(np.float64) @ w_p.astype(np.float64).T)
    logit = z @ v_p.astype(np.float64).T
    loc = 1.0 / (1.0 + np.exp(-logit))
    return loc[:, 0] * float(N - 1)


def _host_prep(c_t, w_p, v_p):
    """Plans the batch->(core, slot) permutation and slot geometry.

    Returns (perm, slot_geom, g_pad) where perm[c*BL + i] is the original
    batch index at core c slot i, slot_geom[i] = (A_i, EW_i), and
    g_pad[b_orig] holds the gaussian weights placed at the batch's offset
    within its slot frame (zero elsewhere).
    """
    p_t = _predict_host(c_t, w_p, v_p)
    p = np.rint(p_t).astype(np.int64)
    cs = p - HALF  # window start column in q_i's last dim
    assert cs.min() >= 0 and cs.max() + WIN <= N, (
        "window out of bounds; NaN-padding path not implemented"
    )

    order = np.argsort(cs, kind="stable")  # sorted batch ids
    # slot i <- sorted ranks [8i, 8i+8), distributed one per core
    perm = np.empty(B, np.int64)
    slot_geom = []
    for i in range(BL):
        grp = order[i * NCORES : (i + 1) * NCORES]
        for c in range(NCORES):
            perm[c * BL + i] = grp[c]
        lo = int(cs[grp].min()) // ALIGN * ALIGN
        hi = int(cs[grp].max()) + WIN
        ew = -((lo - hi) // ALIGN) * ALIGN  # ceil to ALIGN
        ew = min(ew, N - lo)
        slot_geom.append((lo, ew))

    ew_max = max(ew for _, ew in slot_geom)
    w = np.arange(WIN, dtype=np.float64)
    x = (cs[:, None] + w[None, :] - p_t[:, None]) / float(HALF)
    g = np.exp(-2.0 * x * x).astype(np.float32)
    g_pad = np.zeros((B, ew_max), np.float32)
    for i in range(BL):
        a_i, ew_i = slot_geom[i]
        for c in range(NCORES):
            b = perm[c * BL + i]
            r = int(cs[b]) - a_i
            assert 0 <= r and r + WIN <= ew_i
            g_pad[b, r : r + WIN] = g[b]
    return perm, tuple(slot_geom), g_pad


def _make_in_maps(q_i, c_t, w_p, v_p):
    q_i = np.asarray(q_i, dtype=np.float32)
    perm, slot_geom, g_pad = _host_prep(
        np.asarray(c_t, np.float32),
        np.asarray(w_p, np.float32),
        np.asarray(v_p, np.float32),
    )
    in_maps = []
    for c in range(NCORES):
        ids = perm[c * BL : (c + 1) * BL]
        in_maps.append(
            {
                "qs": np.ascontiguousarray(q_i[ids]),
                "gb": np.ascontiguousarray(g_pad[ids]),
            }
        )
    return perm, slot_geom, in_maps


def _untangle_out(raw):
    """[128, BL*QC] device layout -> [BL, Q]: out[p, i*QC+qc] = s_t[i, qc*128+p]."""
    return raw.reshape(128, BL, QC).transpose(1, 2, 0).reshape(BL, Q)


def kernel(q_i, c_t, w_a, w_p, v_p, window):
    assert int(window) == WIN
    from concourse.bass_utils import run_bass_kernel_spmd

    perm, slot_geom, in_maps = _make_in_maps(q_i, c_t, w_p, v_p)
    nc = _get_nc(slot_geom)
    res = run_bass_kernel_spmd(nc, in_maps, core_ids=list(range(NCORES)))
    permuted = np.concatenate(
        [_untangle_out(r["out"]) for r in res.results], axis=0
    )
    out = np.empty_like(permuted)
    out[perm] = permuted
    return out



# revision 9
# speedup vs baseline: 2.1588x; 2.1588x over previous
"""LocalAttention1d Trainium2 kernel (v3: host-windowed bf16 + PE matvec).

Math note: the reference applies softmax over a singleton axis
(softmax(a_t[..., None], axis=2)), which is exactly 1.0 for finite scores,
so the Luong-score path (the two big einsums over w_a) cancels out of the
output. The output reduces exactly to

    s_t[b, q] = sum_w exp(-s_exp[b, w]) * q_i[b, q, p[b] - 128 + w]

with p = round(p_t) from the predictive-alignment network, provided the
window [p-128, p+128) stays in bounds (guaranteed by the tiny v_p init; we
assert it). The tiny predictive network (c_t @ w_p.T -> tanh -> @ v_p.T ->
sigmoid, ~0.1% of the FLOPs) is evaluated on host in float64 to pick the
integer window positions.

Device strategy (pure data parallel, one fully static shape-only NEFF run
SPMD on 8 cores, 8 batches per core): the host extracts each batch's exact
256-column window, transposes it to [window, Q] and casts to bf16 — half
the HBM bytes of f32 and 2KB-contiguous rows, and it puts the window axis
on SBUF partitions so the whole gaussian-weighted reduction becomes PE
matvecs: out[1, Q] = g[256]ᵀ · win[256, Q], accumulated over the two
128-row K-chunks in PSUM (fp32). Four batches share each PSUM bank at
partition offsets {0,32,64,96} (the legal tile positions for M=1), so the
result drains with four strided DMAs. DVE/Pool/most of ACT stay idle; the
kernel is DMA-bound at ~4.2MB/core with PE comfortably underneath.

bf16 numerics: quantizing q (and g) to bf16 adds ~0.1-0.2% rms relative
error to a sum whose terms are exact otherwise (PSUM accumulates fp32) —
two orders of magnitude inside the 2e-2 gate.
"""

import numpy as np

B, Q, N = 64, 1024, 2048
WIN = 256
HALF = WIN // 2  # 128
KC = WIN // 128  # 2 contraction chunks of 128
NCORES = 8
BL = B // NCORES  # batches per core

_NC_CACHE = {}


def _build_nc():
    import concourse.tile as tile
    from concourse import bacc, mybir

    f32 = mybir.dt.float32
    bf16 = mybir.dt.bfloat16
    nc = bacc.Bacc(
        "TRN2", target_bir_lowering=False, debug=False, num_devices=NCORES
    )
    qw = nc.dram_tensor("qw", [BL, WIN, Q], bf16, kind="ExternalInput")
    gv = nc.dram_tensor("gv", [128, BL * KC], bf16, kind="ExternalInput")
    out = nc.dram_tensor("out", [BL, Q], f32, kind="ExternalOutput")

    # [128, BL, KC, Q]: partition = w % 128, free = (batch, k-chunk, q)
    qwa = qw.ap().rearrange("i (c p) q -> p i c q", p=128)

    with tile.TileContext(nc) as tc:
        with (
            tc.tile_pool(name="gpool", bufs=1) as gpool,
            tc.tile_pool(name="wpool", bufs=BL) as wpool,
            tc.tile_pool(name="psum", bufs=4, space="PSUM") as psum,
        ):
            gt = gpool.tile([128, BL * KC], bf16)
            nc.sync.dma_start(gt[:, :], gv.ap())
            # PSUM can't be DMA'd; drain banks to SBUF on the (otherwise
            # idle) scalar engine, keeping partitions {0,64} in place.
            acc = gpool.tile([128, 2 * BL // 2, 512], f32, name="acc")
            wts = []
            for i in range(BL):
                wt = wpool.tile([128, KC, Q], bf16, tag="wt")
                nc.sync.dma_start(wt[:, :, :], qwa[:, i])
                wts.append(wt)
            # banks[2*g + h]: batches 2g, 2g+1 land at partitions 0 and 64
            # (the only legal PSUM base partitions besides 32), q-half h;
            # PE accumulates the two K-chunks in PSUM fp32. 8 banks exactly.
            banks = [
                psum.tile([128, 512], f32, tag="bk", name=f"bk{k}")
                for k in range(2 * BL // 2)
            ]
            for i in range(BL):
                grp, r = divmod(i, 2)
                for c in range(KC):
                    col = i * KC + c
                    for h in range(2):
                        nc.tensor.matmul(
                            banks[2 * grp + h][64 * r : 64 * r + 1, :],
                            gt[:, col : col + 1],
                            wts[i][:, c, 512 * h : 512 * (h + 1)],
                            start=(c == 0),
                            stop=(c == KC - 1),
                        )
            # engines can't stride partitions: one copy per (bank, row),
            # split across the otherwise-idle scalar and vector engines.
            for k in range(2 * BL // 2):
                nc.scalar.copy(acc[0:1, k, :], banks[k][0:1, :])
                nc.vector.tensor_scalar_mul(acc[64:65, k, :], banks[k][64:65, :], 1.0)
            # out[2g+r, 512h+q2] = acc[64r, 2g+h, q2]: one strided DMA.
            oacc = out.ap().rearrange("(g r) (h q) -> r g h q", r=2, h=2)
            nc.sync.dma_start(oacc, acc[0:128:64, :, :])
    nc.compile()
    return nc


def _get_nc():
    if "nc" not in _NC_CACHE:
        _NC_CACHE["nc"] = _build_nc()
    return _NC_CACHE["nc"]


def _predict_host(c_t, w_p, v_p):
    """float64 replica of sigmoid(tanh(c_t @ w_p.T) @ v_p.T) * (N+1-2)."""
    z = np.tanh(c_t.astype(np.float64) @ w_p.astype(np.float64).T)
    logit = z @ v_p.astype(np.float64).T
    loc = 1.0 / (1.0 + np.exp(-logit))
    return loc[:, 0] * float(N - 1)


def _prepare(q_i, c_t, w_p, v_p):
    """Window positions + per-core in_maps (bf16 transposed windows)."""
    import ml_dtypes

    bf16 = ml_dtypes.bfloat16
    q_i = np.asarray(q_i, np.float32)
    p_t = _predict_host(
        np.asarray(c_t, np.float32),
        np.asarray(w_p, np.float32),
        np.asarray(v_p, np.float32),
    )
    p = np.rint(p_t).astype(np.int64)
    cs = p - HALF  # window start column in q_i's last dim
    assert cs.min() >= 0 and cs.max() + WIN <= N, (
        "window out of bounds; NaN-padding path not implemented"
    )
    w = np.arange(WIN, dtype=np.float64)
    x = (cs[:, None] + w[None, :] - p_t[:, None]) / float(HALF)
    g = np.exp(-2.0 * x * x)  # (B, WIN)

    in_maps = []
    for c in range(NCORES):
        qw = np.empty((BL, WIN, Q), bf16)
        for i in range(BL):
            b = c * BL + i
            qw[i] = q_i[b, :, cs[b] : cs[b] + WIN].astype(bf16).T
        gcore = g[c * BL : (c + 1) * BL].astype(bf16)  # [BL, WIN]
        gvc = np.ascontiguousarray(
            gcore.reshape(BL, KC, 128).transpose(2, 0, 1).reshape(128, BL * KC)
        )
        in_maps.append({"qw": qw, "gv": gvc})
    return in_maps


def _assemble(results):
    return np.concatenate([r["out"] for r in results], axis=0)


def kernel(q_i, c_t, w_a, w_p, v_p, window):
    assert int(window) == WIN
    from concourse.bass_utils import run_bass_kernel_spmd

    in_maps = _prepare(q_i, c_t, w_p, v_p)
    nc = _get_nc()
    res = run_bass_kernel_spmd(nc, in_maps, core_ids=list(range(NCORES)))
    return _assemble(res.results)
